# revision 5
# baseline (speedup 1.0000x reference)
"""GCN model (3x GCNConv + LayerNorm + ReLU, mean-pool, 2-layer MLP head)
as a Bass SPMD kernel on 8 Trainium2 NeuronCores.

Sharding: nodes (and their incident edges, keyed by dst) are partitioned into
8 contiguous blocks. Each core computes y = dinv * (x @ W) for its block,
an AllGather replicates y, then each core aggregates messages for its dst
block with dma_gather + selection-matrix matmuls, applies LayerNorm + ReLU,
and finally segment-mean-pool partial sums are AllReduced before a tiny MLP.
"""

import math

import numpy as np

import concourse.bass as bass
import concourse.bacc as bacc
import concourse.tile as tile
import concourse.mybir as mybir
from concourse.bass_utils import run_bass_kernel_spmd
from concourse.library_config import mlp as mlp_lib

F32 = mybir.dt.float32
I16 = mybir.dt.int16
AF = mybir.ActivationFunctionType
OP = mybir.AluOpType

P = 128


class GCNConfig:
    def __init__(self, N=50000, E=800000, F_IN=128, H=256, G=64, A=8, OUT=1,
                 M=8):
        assert F_IN % P == 0 and H % P == 0
        self.N, self.E, self.F_IN, self.H, self.G, self.A, self.OUT, self.M = \
            N, E, F_IN, H, G, A, OUT, M
        self.NL = N // M                      # nodes per core (unpadded)
        assert self.NL * M == N
        self.T = (self.NL + P - 1) // P       # dst tiles per core
        self.NP = self.T * P                  # padded nodes per core
        self.NT = self.NP * M                 # padded total rows in y_full
        assert self.NT % 2 == 0
        self.HALF = self.NT // 2              # half-table rows (int16 range)
        assert self.HALF < 32768
        assert self.NP * (M // 2) == self.HALF  # halves align to core blocks
        self.KH = H // P                      # k-tiles per matmul (and F_IN)
        self.KF = F_IN // P


CFG = GCNConfig()


def _wrap_idx16(vals):
    """[n] int16 -> [128, n//16] in the 16-partition-wrapped, 8x-replicated
    layout dma_gather expects (element i at [i % 16, i // 16])."""
    n = vals.shape[0]
    assert n % 16 == 0
    arr = vals.reshape(n // 16, 16).T.astype(np.int16)   # [16, n/16]
    return np.tile(arr, (8, 1))                          # [128, n/16]


def preprocess(cfg, x, edge_index, batch):
    """Host-side sharding: per-core gather indices, dst slots, norm factors,
    pooling matrices. Returns (per_core dict list, meta dict)."""
    N, E, M, NL, NP, T, HALF = (cfg.N, cfg.E, cfg.M, cfg.NL, cfg.NP, cfg.T,
                                cfg.HALF)
    src = np.asarray(edge_index[0], dtype=np.int64)
    dst = np.asarray(edge_index[1], dtype=np.int64)
    batch = np.asarray(batch, dtype=np.int64)

    deg = np.bincount(dst, minlength=N).astype(np.float64) + 1.0
    dinv = (1.0 / np.sqrt(deg)).astype(np.float32)

    # self-loops as ordinary edges
    loop = np.arange(N, dtype=np.int64)
    src2 = np.concatenate([src, loop])
    dst2 = np.concatenate([dst, loop])

    core = dst2 // NL
    dloc = dst2 % NL
    tile_id = dloc // P
    dslot_v = (dloc % P).astype(np.float32)
    srcp = (src2 // NL) * NP + (src2 % NL)          # padded global row
    half = (srcp >= HALF).astype(np.int64)
    idx16v = (srcp - HALF * half).astype(np.int64)

    # sort key: (core, tile, half) -> contiguous blocks
    key = ((core * T + tile_id) * 2 + half)
    order = np.argsort(key, kind="stable")
    key_s = key[order]
    idx16_s = idx16v[order]
    dslot_s = dslot_v[order]

    nkeys = M * T * 2
    counts = np.bincount(key_s, minlength=nkeys).reshape(M, T, 2)
    starts = np.zeros(nkeys + 1, dtype=np.int64)
    np.cumsum(counts.reshape(-1), out=starts[1:])

    # capacities shared across cores (program is SPMD)
    J = np.ceil(counts.max(axis=0) / P).astype(np.int64)     # [T, 2]
    JMAX = int(J.sum(axis=1).max())

    # pack per-core arrays
    W16 = int(J.sum() * 8)        # idx16 free-dim (int16 cols)
    WD = int(J.sum())             # dslot free-dim (f32 cols, J per block)
    per_core = []
    for c in range(M):
        idx16 = np.zeros((P, W16), np.int16)
        dslot = np.full((P, WD), 300.0, np.float32)
        o16 = 0
        od = 0
        for t in range(T):
            for h in range(2):
                Jth = int(J[t, h])
                if Jth == 0:
                    continue
                Cap = Jth * P
                k = (c * T + t) * 2 + h
                s, e = starts[k], starts[k + 1]
                n = e - s
                vi = np.zeros(Cap, np.int64)
                vd = np.full(Cap, 300.0, np.float32)
                vi[:n] = idx16_s[s:e]
                vd[:n] = dslot_s[s:e]
                idx16[:, o16:o16 + Jth * 8] = _wrap_idx16(vi)
                dslot[:, od:od + Jth] = vd.reshape(Jth, P).T
                o16 += Jth * 8
                od += Jth
        # dinv (padded, pad rows -> 0 so pad y rows are exactly zero)
        dpad = np.zeros(NP, np.float32)
        dpad[:NL] = dinv[c * NL:(c + 1) * NL]
        dinvT = dpad.reshape(T, P).T.copy()              # [128, T]
        # pooling one-hot [128, T, G]
        pm = np.zeros((NP, cfg.G), np.float32)
        pm[np.arange(NL), batch[c * NL:(c + 1) * NL]] = 1.0
        poolm = pm.reshape(T, P, cfg.G).transpose(1, 0, 2).copy()
        # x^T slab [F_IN, NP]
        xT = np.zeros((cfg.F_IN, NP), np.float32)
        xT[:, :NL] = np.asarray(x[c * NL:(c + 1) * NL], np.float32).T
        per_core.append(dict(idx16=idx16, dslot=dslot, dinvT=dinvT,
                             poolm=poolm.reshape(P, T * cfg.G), xT=xT))

    meta = dict(J=J, JMAX=JMAX, W16=W16, WD=WD)
    return per_core, meta


def build_program(cfg, meta, gamma_trivial, beta_trivial, msg_bf16=False):
    MD = mybir.dt.bfloat16 if msg_bf16 else F32
    N, M, T, NP, NT, HALF, H, G, A = (cfg.N, cfg.M, cfg.T, cfg.NP, cfg.NT,
                                      cfg.HALF, cfg.H, cfg.G, cfg.A)
    J = meta["J"]
    JMAX = meta["JMAX"]
    KH, KF = cfg.KH, cfg.KF

    nc = bacc.Bacc("TRN2", target_bir_lowering=False, debug=False,
                   num_devices=M)

    def din(name, shape, dt=F32):
        return nc.dram_tensor(name, shape, dt, kind="ExternalInput").ap()

    xT_ap = din("xT", [cfg.F_IN, NP])
    idx16_ap = din("idx16", [P, meta["W16"]], I16)
    dslot_ap = din("dslot", [P, meta["WD"]], MD)
    dinvT_ap = din("dinvT", [P, T])
    poolm_ap = din("poolm", [P, T * G])
    bgb_ap = din("bgb", [P, 9, H])
    iota_ap = din("iota_in", [P, P], MD)
    ident_ap = din("ident_in", [P, P])
    W1_ap = din("W1", [cfg.F_IN, H])
    W2_ap = din("W2", [H, H])
    W3_ap = din("W3", [H, H])
    fc1_ap = din("fc1aug", [3 * P, H])
    attr_ap = din("attraug", [P, G])
    invc_ap = din("invc", [G, 1])
    fcw2_ap = din("fcw2row", [1, H])
    fcb2_ap = din("fcb2col", [G, 1])
    out_ap = nc.dram_tensor("out", [G, cfg.OUT], F32,
                            kind="ExternalOutput").ap()

    y_cc = nc.dram_tensor("y_cc", [NP, H], MD)
    y_full = nc.dram_tensor("y_full", [NT, H], MD, addr_space="Shared")
    pool_in = nc.dram_tensor("pool_in", [G, H], F32)
    pool_out = nc.dram_tensor("pool_out", [G, H], F32, addr_space="Shared")

    rg = [list(range(M))]

    with tile.TileContext(nc) as tc:
        with tc.tile_pool(name="const", bufs=1) as cst, \
             tc.tile_pool(name="sbw", bufs=3) as sbw, \
             tc.tile_pool(name="msgp", bufs=2) as msgp, \
             tc.tile_pool(name="sp", bufs=4) as sp, \
             tc.tile_pool(name="small", bufs=8) as small, \
             tc.tile_pool(name="psp_a", bufs=2, space="PSUM") as psp_a, \
             tc.tile_pool(name="psp_y", bufs=2, space="PSUM") as psp_y, \
             tc.tile_pool(name="pst", bufs=2, space="PSUM") as pst, \
             tc.tile_pool(name="ptail", bufs=2, space="PSUM") as ptail:

            nc.gpsimd.load_library(mlp_lib)

            # ---- constants
            idx16_sb = cst.tile([P, meta["W16"]], I16)
            nc.sync.dma_start(out=idx16_sb[:], in_=idx16_ap[:])
            dslot_sb = cst.tile([P, meta["WD"]], MD)
            nc.sync.dma_start(out=dslot_sb[:], in_=dslot_ap[:])
            dinv_sb = cst.tile([P, T], F32)
            nc.sync.dma_start(out=dinv_sb[:], in_=dinvT_ap[:])
            poolm_sb = cst.tile([P, T, G], F32)
            nc.sync.dma_start(out=poolm_sb[:],
                              in_=poolm_ap[:].rearrange("p (t g) -> p t g", g=G))
            bgb_sb = cst.tile([P, 9, H], F32)
            nc.sync.dma_start(out=bgb_sb[:], in_=bgb_ap[:])
            iota_sb = cst.tile([P, P], MD)
            nc.sync.dma_start(out=iota_sb[:], in_=iota_ap[:])
            ident_sb = cst.tile([P, P], F32)
            nc.sync.dma_start(out=ident_sb[:], in_=ident_ap[:])
            W1_sb = cst.tile([cfg.F_IN, H], F32)
            nc.sync.dma_start(out=W1_sb[:], in_=W1_ap[:])
            W2_sb = cst.tile([P, KH, H], F32)
            nc.sync.dma_start(out=W2_sb[:],
                              in_=W2_ap[:].rearrange("(k p) h -> p k h", p=P))
            W3_sb = cst.tile([P, KH, H], F32)
            nc.sync.dma_start(out=W3_sb[:],
                              in_=W3_ap[:].rearrange("(k p) h -> p k h", p=P))
            fc1_sb = cst.tile([P, 3, H], F32)
            nc.sync.dma_start(out=fc1_sb[:],
                              in_=fc1_ap[:].rearrange("(k p) h -> p k h", p=P))
            attr_sb = cst.tile([P, G], F32)
            nc.sync.dma_start(out=attr_sb[:], in_=attr_ap[:])
            invc_sb = cst.tile([G, 1], F32)
            nc.sync.dma_start(out=invc_sb[:], in_=invc_ap[:])
            fcw2_sb = cst.tile([1, H], F32)
            nc.sync.dma_start(out=fcw2_sb[:], in_=fcw2_ap[:])
            fcb2_sb = cst.tile([G, 1], F32)
            nc.sync.dma_start(out=fcb2_sb[:], in_=fcb2_ap[:])

            eps_sb = cst.tile([P, 1], F32)
            nc.vector.memset(eps_sb[:], 1e-5)
            ones_sb = cst.tile([1, G], F32)
            nc.vector.memset(ones_sb[:], 1.0)
            pool_acc = cst.tile([G, H], F32)
            nc.vector.memset(pool_acc[:], 0.0)
            hT_sb = cst.tile([P, T * KH, P], F32)

            # gpsimd registers for num_idxs (reuse per distinct value)
            regs = {}
            for v in sorted({int(J[t, h]) * P for t in range(T)
                             for h in range(2) if J[t, h] > 0}):
                regs[v] = nc.gpsimd.to_reg(v)

            # block offsets into idx16/dslot slabs
            o16 = np.zeros((T, 2), np.int64)
            od = np.zeros((T, 2), np.int64)
            acc16 = 0
            accd = 0
            for t in range(T):
                for h in range(2):
                    o16[t, h] = acc16
                    od[t, h] = accd
                    acc16 += int(J[t, h]) * 8
                    accd += int(J[t, h])

            Wsb = [W1_sb, W2_sb, W3_sb]

            for L in range(3):
                # ---------- phase A: y = dinv * (h @ W) ----------
                for t in range(T):
                    psy = psp_y.tile([P, H], F32, tag="psy")
                    if L == 0:
                        xt = sbw.tile([P, P], F32, tag="xt")
                        nc.sync.dma_start(out=xt[:],
                                          in_=xT_ap[:, t * P:(t + 1) * P])
                        nc.tensor.matmul(psy[:], lhsT=xt[:], rhs=W1_sb[:],
                                         start=True, stop=True)
                    else:
                        for kk in range(KH):
                            nc.tensor.matmul(
                                psy[:], lhsT=hT_sb[:, t * KH + kk, :],
                                rhs=Wsb[L][:, kk, :],
                                start=(kk == 0), stop=(kk == KH - 1))
                    ysb = sbw.tile([P, H], MD, tag="y")
                    nc.vector.tensor_scalar_mul(out=ysb[:], in0=psy[:],
                                                scalar1=dinv_sb[:, t:t + 1])
                    nc.sync.dma_start(out=y_cc[t * P:(t + 1) * P, :],
                                      in_=ysb[:])

                # ---------- exchange ----------
                nc.gpsimd.collective_compute(
                    "AllGather", OP.bypass, replica_groups=rg,
                    ins=[y_cc[:]], outs=[y_full[:]])

                # ---------- aggregation per dst tile ----------
                for t in range(T):
                    Jl, Jh = int(J[t, 0]), int(J[t, 1])
                    Jt = Jl + Jh
                    msg = msgp.tile([P, JMAX, H], MD, tag="msg")
                    if Jl > 0:
                        nc.gpsimd.dma_gather(
                            out_ap=msg[:, :Jl, :], in_ap=y_full[:HALF, :],
                            idxs_ap=idx16_sb[:, int(o16[t, 0]):int(o16[t, 0]) + Jl * 8],
                            num_idxs=Jl * P, num_idxs_reg=regs[Jl * P],
                            elem_size=H, single_packet=False)
                    if Jh > 0:
                        nc.gpsimd.dma_gather(
                            out_ap=msg[:, Jl:Jt, :], in_ap=y_full[HALF:, :],
                            idxs_ap=idx16_sb[:, int(o16[t, 1]):int(o16[t, 1]) + Jh * 8],
                            num_idxs=Jh * P, num_idxs_reg=regs[Jh * P],
                            elem_size=H, single_packet=False)
                    ps = psp_a.tile([P, H], F32, tag="agg")
                    for j in range(Jt):
                        jj = int(od[t, 0]) + j if j < Jl else int(od[t, 1]) + (j - Jl)
                        S = sp.tile([P, P], MD, tag="S")
                        nc.vector.tensor_tensor(
                            out=S[:], in0=iota_sb[:],
                            in1=dslot_sb[:, jj:jj + 1].to_broadcast([P, P]),
                            op=OP.is_equal)
                        nc.tensor.matmul(ps[:], lhsT=S[:], rhs=msg[:, j, :],
                                         start=(j == 0), stop=(j == Jt - 1))

                    # ---------- evict + bias + LN + relu ----------
                    tt = sbw.tile([P, H], F32, tag="tt")
                    nc.vector.tensor_scalar_mul(out=tt[:], in0=ps[:],
                                                scalar1=dinv_sb[:, t:t + 1])
                    nc.vector.tensor_add(out=tt[:], in0=tt[:],
                                         in1=bgb_sb[:, 3 * L + 0, :])
                    stats = small.tile([P, 6], F32, tag="stats")
                    nc.vector.bn_stats(out=stats[:], in_=tt[:])
                    mv = small.tile([P, 2], F32, tag="mv")
                    nc.vector.bn_aggr(out=mv[:], in_=stats[:])
                    rstd = small.tile([P, 1], F32, tag="rstd")
                    nc.scalar.activation(out=rstd[:], in_=mv[:, 1:2],
                                         func=AF.Sqrt, bias=eps_sb[:],
                                         scale=1.0)
                    nc.vector.reciprocal(out=rstd[:], in_=rstd[:])
                    nc.vector.tensor_scalar(
                        out=tt[:], in0=tt[:], scalar1=mv[:, 0:1],
                        scalar2=rstd[:], op0=OP.subtract, op1=OP.mult)
                    if not gamma_trivial:
                        nc.vector.tensor_mul(out=tt[:], in0=tt[:],
                                             in1=bgb_sb[:, 3 * L + 1, :])
                    if not beta_trivial:
                        nc.vector.tensor_add(out=tt[:], in0=tt[:],
                                             in1=bgb_sb[:, 3 * L + 2, :])
                    h_t = sbw.tile([P, H], F32, tag="h")
                    nc.scalar.activation(out=h_t[:], in_=tt[:], func=AF.Relu)

                    if L < 2:
                        for kk in range(KH):
                            pt = pst.tile([P, P], F32, tag="pt")
                            nc.tensor.transpose(
                                out=pt[:], in_=h_t[:, kk * P:(kk + 1) * P],
                                identity=ident_sb[:])
                            nc.vector.tensor_copy(
                                out=hT_sb[:, t * KH + kk, :], in_=pt[:])
                    else:
                        pp = ptail.tile([G, H], F32, tag="tail")
                        nc.tensor.matmul(pp[:], lhsT=poolm_sb[:, t, :],
                                         rhs=h_t[:], start=True, stop=True)
                        nc.vector.tensor_add(out=pool_acc[:], in0=pool_acc[:],
                                             in1=pp[:])

            # ---------- pooled mean + MLP head ----------
            nc.sync.dma_start(out=pool_in[:], in_=pool_acc[:])
            nc.gpsimd.collective_compute(
                "AllReduce", OP.add, replica_groups=rg,
                ins=[pool_in[:]], outs=[pool_out[:]])
            pooled = sbw.tile([G, H], F32, tag="pooled")
            nc.sync.dma_start(out=pooled[:], in_=pool_out[:])
            nc.vector.tensor_scalar_mul(out=pooled[:], in0=pooled[:],
                                        scalar1=invc_sb[:])
            zt = sbw.tile([P, KH, G], F32, tag="zt")
            for kk in range(KH):
                pz = ptail.tile([P, G], F32, tag="tail")
                nc.tensor.transpose(out=pz[:], in_=pooled[:, kk * P:(kk + 1) * P],
                                    identity=ident_sb[:G, :G])
                nc.vector.tensor_copy(out=zt[:, kk, :], in_=pz[:])
            ups = ptail.tile([G, H], F32, tag="tail")
            nc.tensor.matmul(ups[:], lhsT=zt[:, 0, :], rhs=fc1_sb[:, 0, :],
                             start=True, stop=False)
            nc.tensor.matmul(ups[:], lhsT=zt[:, 1, :], rhs=fc1_sb[:, 1, :],
                             start=False, stop=False)
            nc.tensor.matmul(ups[:], lhsT=attr_sb[:], rhs=fc1_sb[:, 2, :],
                             start=False, stop=True)
            r = sbw.tile([G, H], F32, tag="r")
            nc.scalar.activation(out=r[:], in_=ups[:], func=AF.Relu)
            wps = ptail.tile([G, H], F32, tag="tail")
            nc.tensor.matmul(wps[:], lhsT=ones_sb[:], rhs=fcw2_sb[:],
                             start=True, stop=True)
            rr = sbw.tile([G, H], F32, tag="rr")
            nc.vector.tensor_mul(out=rr[:], in0=r[:], in1=wps[:])
            o = small.tile([G, 1], F32, tag="o")
            nc.vector.tensor_reduce(out=o[:], in_=rr[:],
                                    axis=mybir.AxisListType.X, op=OP.add)
            nc.vector.tensor_scalar_add(out=o[:], in0=o[:],
                                        scalar1=fcb2_sb[:])
            nc.sync.dma_start(out=out_ap[:], in_=o[:])

    nc.compile()
    return nc


def make_in_maps(cfg, inputs, per_core, msg_bf16=False):
    """Build the per-core input maps from full inputs + preprocessed arrays."""
    H, G, A = cfg.H, cfg.G, cfg.A
    f = lambda a: np.ascontiguousarray(np.asarray(a, np.float32))
    W1, b1 = f(inputs["W1"]), f(inputs["b1"])
    W2, b2 = f(inputs["W2"]), f(inputs["b2"])
    W3, b3 = f(inputs["W3"]), f(inputs["b3"])
    g1, be1 = f(inputs["g1"]), f(inputs["be1"])
    g2, be2 = f(inputs["g2"]), f(inputs["be2"])
    g3, be3 = f(inputs["g3"]), f(inputs["be3"])
    fcW1, fcb1 = f(inputs["fcW1"]), f(inputs["fcb1"])
    fcW2, fcb2 = f(inputs["fcW2"]), f(inputs["fcb2"])
    graph_attr = f(inputs["graph_attr"]).reshape(-1, A)
    batch = np.asarray(inputs["batch"], np.int64)

    bgb = np.zeros((P, 9, H), np.float32)
    for i, v in enumerate([b1, g1, be1, b2, g2, be2, b3, g3, be3]):
        bgb[:, i, :] = v[None, :]
    fc1aug = np.zeros((3 * P, H), np.float32)
    fc1aug[:H, :] = fcW1[:H, :]
    fc1aug[2 * P:2 * P + A, :] = fcW1[H:H + A, :]
    fc1aug[2 * P + A, :] = fcb1
    attraug = np.zeros((P, G), np.float32)
    attraug[:A, :] = graph_attr.T
    attraug[A, :] = 1.0
    cnt = np.bincount(batch, minlength=G).astype(np.float32)
    invc = (1.0 / np.maximum(cnt, 1.0)).reshape(G, 1).astype(np.float32)
    fcw2row = fcW2[:, 0].reshape(1, H).copy()
    fcb2col = np.full((G, 1), fcb2[0], np.float32)
    iota_in = np.tile(np.arange(P, dtype=np.float32), (P, 1))
    if msg_bf16:
        import ml_dtypes
        iota_in = iota_in.astype(ml_dtypes.bfloat16)
    ident_in = np.eye(P, dtype=np.float32)

    shared = dict(bgb=bgb, iota_in=iota_in, ident_in=ident_in, W1=W1, W2=W2,
                  W3=W3, fc1aug=fc1aug, attraug=attraug, invc=invc,
                  fcw2row=fcw2row, fcb2col=fcb2col)
    in_maps = []
    for c in range(cfg.M):
        m = dict(shared)
        m.update(per_core[c])
        if msg_bf16:
            import ml_dtypes
            m["dslot"] = m["dslot"].astype(ml_dtypes.bfloat16)
        in_maps.append(m)
    return in_maps


_CACHE = {}


def _get_program(cfg, meta, gamma_trivial, beta_trivial, msg_bf16=False):
    key = (tuple(meta["J"].reshape(-1).tolist()), gamma_trivial, beta_trivial,
           msg_bf16)
    if key not in _CACHE:
        _CACHE[key] = build_program(cfg, meta, gamma_trivial, beta_trivial,
                                    msg_bf16)
    return _CACHE[key]


def run(cfg, inputs, nc=None, return_nc=False, msg_bf16=False):
    per_core, meta = preprocess(cfg, inputs["x"], inputs["edge_index"],
                                inputs["batch"])
    gamma_trivial = all(np.allclose(np.asarray(inputs[k]), 1.0)
                        for k in ("g1", "g2", "g3"))
    beta_trivial = all(np.allclose(np.asarray(inputs[k]), 0.0)
                       for k in ("be1", "be2", "be3"))
    if nc is None:
        nc = _get_program(cfg, meta, gamma_trivial, beta_trivial, msg_bf16)
    in_maps = make_in_maps(cfg, inputs, per_core, msg_bf16)
    res = None
    for attempt in range(3):
        try:
            res = run_bass_kernel_spmd(nc, in_maps, list(range(cfg.M)))
            break
        except Exception:
            # a previously crashed run can leave the device unrecoverable
            # for exactly one execution; retry clears it
            if attempt == 2:
                raise
    out = res.results[0]["out"].astype(np.float32)
    if return_nc:
        return out, nc
    return out


def kernel(**inputs) -> np.ndarray:
    return run(CFG, inputs)


# revision 6
# speedup vs baseline: 175.4778x; 175.4778x over previous
"""GCN model (3x GCNConv + LayerNorm + ReLU, mean-pool, 2-layer MLP head)
as a Bass SPMD kernel on 8 Trainium2 NeuronCores.

Sharding: nodes (and their incident edges, keyed by dst) are partitioned into
8 contiguous blocks. Each core computes y = dinv * (x @ W) for its block,
an AllGather replicates y, then each core aggregates messages for its dst
block with dma_gather + selection-matrix matmuls, applies LayerNorm + ReLU,
and finally segment-mean-pool partial sums are AllReduced before a tiny MLP.
"""

import math

import numpy as np

import concourse.bass as bass
import concourse.bacc as bacc
import concourse.tile as tile
import concourse.mybir as mybir
from concourse.bass_utils import run_bass_kernel_spmd
from concourse.library_config import mlp as mlp_lib

F32 = mybir.dt.float32
I16 = mybir.dt.int16
AF = mybir.ActivationFunctionType
OP = mybir.AluOpType

P = 128


class GCNConfig:
    def __init__(self, N=50000, E=800000, F_IN=128, H=256, G=64, A=8, OUT=1,
                 M=8):
        assert F_IN % P == 0 and H % P == 0
        self.N, self.E, self.F_IN, self.H, self.G, self.A, self.OUT, self.M = \
            N, E, F_IN, H, G, A, OUT, M
        self.NL = N // M                      # nodes per core (unpadded)
        assert self.NL * M == N
        self.T = (self.NL + P - 1) // P       # dst tiles per core
        self.NP = self.T * P                  # padded nodes per core
        self.NT = self.NP * M                 # padded total rows in y_full
        assert self.NT % 2 == 0
        self.HALF = self.NT // 2              # half-table rows (int16 range)
        assert self.HALF < 32768
        assert self.NP * (M // 2) == self.HALF  # halves align to core blocks
        self.KH = H // P                      # k-tiles per matmul (and F_IN)
        self.KF = F_IN // P


CFG = GCNConfig()


def _wrap_idx16(vals):
    """[n] int16 -> [128, n//16] in the 16-partition-wrapped, 8x-replicated
    layout dma_gather expects (element i at [i % 16, i // 16])."""
    n = vals.shape[0]
    assert n % 16 == 0
    arr = vals.reshape(n // 16, 16).T.astype(np.int16)   # [16, n/16]
    return np.tile(arr, (8, 1))                          # [128, n/16]


def preprocess(cfg, x, edge_index, batch):
    """Host-side sharding: per-core gather indices, dst slots, norm factors,
    pooling matrices. Returns (per_core dict list, meta dict)."""
    N, E, M, NL, NP, T, HALF = (cfg.N, cfg.E, cfg.M, cfg.NL, cfg.NP, cfg.T,
                                cfg.HALF)
    src = np.asarray(edge_index[0], dtype=np.int64)
    dst = np.asarray(edge_index[1], dtype=np.int64)
    batch = np.asarray(batch, dtype=np.int64)

    deg = np.bincount(dst, minlength=N).astype(np.float64) + 1.0
    dinv = (1.0 / np.sqrt(deg)).astype(np.float32)

    # self-loops as ordinary edges
    loop = np.arange(N, dtype=np.int64)
    src2 = np.concatenate([src, loop])
    dst2 = np.concatenate([dst, loop])

    core = dst2 // NL
    dloc = dst2 % NL
    tile_id = dloc // P
    dslot_v = (dloc % P).astype(np.float32)
    srcp = (src2 // NL) * NP + (src2 % NL)          # padded global row
    half = (srcp >= HALF).astype(np.int64)
    idx16v = (srcp - HALF * half).astype(np.int64)

    # sort key: (core, tile, half) -> contiguous blocks
    key = ((core * T + tile_id) * 2 + half)
    order = np.argsort(key, kind="stable")
    key_s = key[order]
    idx16_s = idx16v[order]
    dslot_s = dslot_v[order]

    nkeys = M * T * 2
    counts = np.bincount(key_s, minlength=nkeys).reshape(M, T, 2)
    starts = np.zeros(nkeys + 1, dtype=np.int64)
    np.cumsum(counts.reshape(-1), out=starts[1:])

    # capacities shared across cores (program is SPMD)
    J = np.ceil(counts.max(axis=0) / P).astype(np.int64)     # [T, 2]
    JMAX = int(J.sum(axis=1).max())

    # pack per-core arrays
    W16 = int(J.sum() * 8)        # idx16 free-dim (int16 cols)
    WD = int(J.sum())             # dslot free-dim (f32 cols, J per block)
    per_core = []
    for c in range(M):
        idx16 = np.zeros((P, W16), np.int16)
        dslot = np.full((P, WD), 300.0, np.float32)
        o16 = 0
        od = 0
        for t in range(T):
            for h in range(2):
                Jth = int(J[t, h])
                if Jth == 0:
                    continue
                Cap = Jth * P
                k = (c * T + t) * 2 + h
                s, e = starts[k], starts[k + 1]
                n = e - s
                vi = np.zeros(Cap, np.int64)
                vd = np.full(Cap, 300.0, np.float32)
                vi[:n] = idx16_s[s:e]
                vd[:n] = dslot_s[s:e]
                idx16[:, o16:o16 + Jth * 8] = _wrap_idx16(vi)
                dslot[:, od:od + Jth] = vd.reshape(Jth, P).T
                o16 += Jth * 8
                od += Jth
        # dinv (padded, pad rows -> 0 so pad y rows are exactly zero)
        dpad = np.zeros(NP, np.float32)
        dpad[:NL] = dinv[c * NL:(c + 1) * NL]
        dinvT = dpad.reshape(T, P).T.copy()              # [128, T]
        # pooling one-hot [128, T, G]
        pm = np.zeros((NP, cfg.G), np.float32)
        pm[np.arange(NL), batch[c * NL:(c + 1) * NL]] = 1.0
        poolm = pm.reshape(T, P, cfg.G).transpose(1, 0, 2).copy()
        # x^T slab [F_IN, NP]
        xT = np.zeros((cfg.F_IN, NP), np.float32)
        xT[:, :NL] = np.asarray(x[c * NL:(c + 1) * NL], np.float32).T
        per_core.append(dict(idx16=idx16, dslot=dslot, dinvT=dinvT,
                             poolm=poolm.reshape(P, T * cfg.G), xT=xT))

    meta = dict(J=J, JMAX=JMAX, W16=W16, WD=WD)
    return per_core, meta


def build_program(cfg, meta, gamma_trivial, beta_trivial, msg_bf16=False):
    MD = mybir.dt.bfloat16 if msg_bf16 else F32
    N, M, T, NP, NT, HALF, H, G, A = (cfg.N, cfg.M, cfg.T, cfg.NP, cfg.NT,
                                      cfg.HALF, cfg.H, cfg.G, cfg.A)
    J = meta["J"]
    JMAX = meta["JMAX"]
    KH, KF = cfg.KH, cfg.KF

    nc = bacc.Bacc("TRN2", target_bir_lowering=False, debug=False,
                   num_devices=M, num_swdge_queues=2)

    def din(name, shape, dt=F32):
        return nc.dram_tensor(name, shape, dt, kind="ExternalInput").ap()

    xT_ap = din("xT", [cfg.F_IN, NP])
    idx16_ap = din("idx16", [P, meta["W16"]], I16)
    dslot_ap = din("dslot", [P, meta["WD"]], MD)
    dinvT_ap = din("dinvT", [P, T])
    poolm_ap = din("poolm", [P, T * G])
    bgb_ap = din("bgb", [P, 9, H])
    iota_ap = din("iota_in", [P, P], MD)
    ident_ap = din("ident_in", [P, P])
    W1_ap = din("W1", [cfg.F_IN, H])
    W2_ap = din("W2", [H, H])
    W3_ap = din("W3", [H, H])
    fc1_ap = din("fc1aug", [3 * P, H])
    attr_ap = din("attraug", [P, G])
    invc_ap = din("invc", [G, 1])
    fcw2_ap = din("fcw2row", [1, H])
    fcb2_ap = din("fcb2col", [G, 1])
    out_ap = nc.dram_tensor("out", [G, cfg.OUT], F32,
                            kind="ExternalOutput").ap()

    y_cc = nc.dram_tensor("y_cc", [NP, H], MD)
    y_full = nc.dram_tensor("y_full", [NT, H], MD, addr_space="Shared")
    pool_in = nc.dram_tensor("pool_in", [G, H], F32)
    pool_out = nc.dram_tensor("pool_out", [G, H], F32, addr_space="Shared")

    rg = [list(range(M))]

    with tile.TileContext(nc) as tc:
        with tc.tile_pool(name="const", bufs=1) as cst, \
             tc.tile_pool(name="sbw", bufs=3) as sbw, \
             tc.tile_pool(name="msgp", bufs=2) as msgp, \
             tc.tile_pool(name="sp", bufs=4) as sp, \
             tc.tile_pool(name="small", bufs=8) as small, \
             tc.tile_pool(name="psp_a", bufs=2, space="PSUM") as psp_a, \
             tc.tile_pool(name="psp_y", bufs=2, space="PSUM") as psp_y, \
             tc.tile_pool(name="pst", bufs=2, space="PSUM") as pst, \
             tc.tile_pool(name="ptail", bufs=2, space="PSUM") as ptail:

            nc.gpsimd.load_library(mlp_lib)

            # ---- constants
            idx16_sb = cst.tile([P, meta["W16"]], I16)
            nc.sync.dma_start(out=idx16_sb[:], in_=idx16_ap[:])
            dslot_sb = cst.tile([P, meta["WD"]], MD)
            nc.sync.dma_start(out=dslot_sb[:], in_=dslot_ap[:])
            dinv_sb = cst.tile([P, T], F32)
            nc.sync.dma_start(out=dinv_sb[:], in_=dinvT_ap[:])
            poolm_sb = cst.tile([P, T, G], F32)
            nc.sync.dma_start(out=poolm_sb[:],
                              in_=poolm_ap[:].rearrange("p (t g) -> p t g", g=G))
            bgb_sb = cst.tile([P, 9, H], F32)
            nc.sync.dma_start(out=bgb_sb[:], in_=bgb_ap[:])
            iota_sb = cst.tile([P, P], MD)
            nc.sync.dma_start(out=iota_sb[:], in_=iota_ap[:])
            ident_sb = cst.tile([P, P], F32)
            nc.sync.dma_start(out=ident_sb[:], in_=ident_ap[:])
            W1_sb = cst.tile([cfg.F_IN, H], F32)
            nc.sync.dma_start(out=W1_sb[:], in_=W1_ap[:])
            W2_sb = cst.tile([P, KH, H], F32)
            nc.sync.dma_start(out=W2_sb[:],
                              in_=W2_ap[:].rearrange("(k p) h -> p k h", p=P))
            W3_sb = cst.tile([P, KH, H], F32)
            nc.sync.dma_start(out=W3_sb[:],
                              in_=W3_ap[:].rearrange("(k p) h -> p k h", p=P))
            fc1_sb = cst.tile([P, 3, H], F32)
            nc.sync.dma_start(out=fc1_sb[:],
                              in_=fc1_ap[:].rearrange("(k p) h -> p k h", p=P))
            attr_sb = cst.tile([P, G], F32)
            nc.sync.dma_start(out=attr_sb[:], in_=attr_ap[:])
            invc_sb = cst.tile([G, 1], F32)
            nc.sync.dma_start(out=invc_sb[:], in_=invc_ap[:])
            fcw2_sb = cst.tile([1, H], F32)
            nc.sync.dma_start(out=fcw2_sb[:], in_=fcw2_ap[:])
            fcb2_sb = cst.tile([G, 1], F32)
            nc.sync.dma_start(out=fcb2_sb[:], in_=fcb2_ap[:])

            eps_sb = cst.tile([P, 1], F32)
            nc.vector.memset(eps_sb[:], 1e-5)
            ones_sb = cst.tile([1, G], F32)
            nc.vector.memset(ones_sb[:], 1.0)
            pool_acc = cst.tile([G, H], F32)
            nc.vector.memset(pool_acc[:], 0.0)
            hT_sb = cst.tile([P, T * KH, P], F32)

            # gpsimd registers for num_idxs (reuse per distinct value)
            regs = {}
            for v in sorted({int(J[t, h]) * P for t in range(T)
                             for h in range(2) if J[t, h] > 0}):
                regs[v] = nc.gpsimd.to_reg(v)

            # block offsets into idx16/dslot slabs
            o16 = np.zeros((T, 2), np.int64)
            od = np.zeros((T, 2), np.int64)
            acc16 = 0
            accd = 0
            for t in range(T):
                for h in range(2):
                    o16[t, h] = acc16
                    od[t, h] = accd
                    acc16 += int(J[t, h]) * 8
                    accd += int(J[t, h])

            Wsb = [W1_sb, W2_sb, W3_sb]

            for L in range(3):
                # ---------- phase A: y = dinv * (h @ W) ----------
                for t in range(T):
                    psy = psp_y.tile([P, H], F32, tag="psy")
                    if L == 0:
                        xt = sbw.tile([P, P], F32, tag="xt")
                        nc.sync.dma_start(out=xt[:],
                                          in_=xT_ap[:, t * P:(t + 1) * P])
                        nc.tensor.matmul(psy[:], lhsT=xt[:], rhs=W1_sb[:],
                                         start=True, stop=True)
                    else:
                        for kk in range(KH):
                            nc.tensor.matmul(
                                psy[:], lhsT=hT_sb[:, t * KH + kk, :],
                                rhs=Wsb[L][:, kk, :],
                                start=(kk == 0), stop=(kk == KH - 1))
                    ysb = sbw.tile([P, H], MD, tag="y")
                    nc.scalar.mul(out=ysb[:], in_=psy[:],
                                  mul=dinv_sb[:, t:t + 1])
                    nc.sync.dma_start(out=y_cc[t * P:(t + 1) * P, :],
                                      in_=ysb[:])

                # ---------- exchange ----------
                nc.gpsimd.collective_compute(
                    "AllGather", OP.bypass, replica_groups=rg,
                    ins=[y_cc[:]], outs=[y_full[:]])

                # ---------- aggregation per dst tile ----------
                for t in range(T):
                    Jl, Jh = int(J[t, 0]), int(J[t, 1])
                    Jt = Jl + Jh
                    msg = msgp.tile([P, JMAX, H], MD, tag="msg")
                    if Jl > 0:
                        nc.gpsimd.dma_gather(
                            out_ap=msg[:, :Jl, :], in_ap=y_full[:HALF, :],
                            idxs_ap=idx16_sb[:, int(o16[t, 0]):int(o16[t, 0]) + Jl * 8],
                            num_idxs=Jl * P, num_idxs_reg=regs[Jl * P],
                            elem_size=H, single_packet=False,
                            queue_num=t % 2)
                    if Jh > 0:
                        nc.gpsimd.dma_gather(
                            out_ap=msg[:, Jl:Jt, :], in_ap=y_full[HALF:, :],
                            idxs_ap=idx16_sb[:, int(o16[t, 1]):int(o16[t, 1]) + Jh * 8],
                            num_idxs=Jh * P, num_idxs_reg=regs[Jh * P],
                            elem_size=H, single_packet=False,
                            queue_num=t % 2)
                    ps = psp_a.tile([P, H], F32, tag="agg")
                    for j in range(Jt):
                        jj = int(od[t, 0]) + j if j < Jl else int(od[t, 1]) + (j - Jl)
                        S = sp.tile([P, P], MD, tag="S")
                        nc.vector.tensor_tensor(
                            out=S[:], in0=iota_sb[:],
                            in1=dslot_sb[:, jj:jj + 1].to_broadcast([P, P]),
                            op=OP.is_equal)
                        nc.tensor.matmul(ps[:], lhsT=S[:], rhs=msg[:, j, :],
                                         start=(j == 0), stop=(j == Jt - 1))

                    # ---------- evict + bias + LN + relu ----------
                    tt = sbw.tile([P, H], F32, tag="tt")
                    nc.vector.tensor_scalar_mul(out=tt[:], in0=ps[:],
                                                scalar1=dinv_sb[:, t:t + 1])
                    nc.vector.tensor_add(out=tt[:], in0=tt[:],
                                         in1=bgb_sb[:, 3 * L + 0, :])
                    stats = small.tile([P, 6], F32, tag="stats")
                    nc.vector.bn_stats(out=stats[:], in_=tt[:])
                    mv = small.tile([P, 2], F32, tag="mv")
                    nc.vector.bn_aggr(out=mv[:], in_=stats[:])
                    rstd = small.tile([P, 1], F32, tag="rstd")
                    nc.scalar.activation(out=rstd[:], in_=mv[:, 1:2],
                                         func=AF.Sqrt, bias=eps_sb[:],
                                         scale=1.0)
                    nc.vector.reciprocal(out=rstd[:], in_=rstd[:])
                    nc.vector.tensor_scalar(
                        out=tt[:], in0=tt[:], scalar1=mv[:, 0:1],
                        scalar2=rstd[:], op0=OP.subtract, op1=OP.mult)
                    if not gamma_trivial:
                        nc.vector.tensor_mul(out=tt[:], in0=tt[:],
                                             in1=bgb_sb[:, 3 * L + 1, :])
                    if not beta_trivial:
                        nc.vector.tensor_add(out=tt[:], in0=tt[:],
                                             in1=bgb_sb[:, 3 * L + 2, :])
                    h_t = sbw.tile([P, H], F32, tag="h")
                    nc.scalar.activation(out=h_t[:], in_=tt[:], func=AF.Relu)

                    if L < 2:
                        for kk in range(KH):
                            pt = pst.tile([P, P], F32, tag="pt")
                            nc.tensor.transpose(
                                out=pt[:], in_=h_t[:, kk * P:(kk + 1) * P],
                                identity=ident_sb[:])
                            nc.vector.tensor_copy(
                                out=hT_sb[:, t * KH + kk, :], in_=pt[:])
                    else:
                        pp = ptail.tile([G, H], F32, tag="tail")
                        nc.tensor.matmul(pp[:], lhsT=poolm_sb[:, t, :],
                                         rhs=h_t[:], start=True, stop=True)
                        nc.vector.tensor_add(out=pool_acc[:], in0=pool_acc[:],
                                             in1=pp[:])

            # ---------- pooled mean + MLP head ----------
            nc.sync.dma_start(out=pool_in[:], in_=pool_acc[:])
            nc.gpsimd.collective_compute(
                "AllReduce", OP.add, replica_groups=rg,
                ins=[pool_in[:]], outs=[pool_out[:]])
            pooled = sbw.tile([G, H], F32, tag="pooled")
            nc.sync.dma_start(out=pooled[:], in_=pool_out[:])
            nc.vector.tensor_scalar_mul(out=pooled[:], in0=pooled[:],
                                        scalar1=invc_sb[:])
            zt = sbw.tile([P, KH, G], F32, tag="zt")
            for kk in range(KH):
                pz = ptail.tile([P, G], F32, tag="tail")
                nc.tensor.transpose(out=pz[:], in_=pooled[:, kk * P:(kk + 1) * P],
                                    identity=ident_sb[:G, :G])
                nc.vector.tensor_copy(out=zt[:, kk, :], in_=pz[:])
            ups = ptail.tile([G, H], F32, tag="tail")
            nc.tensor.matmul(ups[:], lhsT=zt[:, 0, :], rhs=fc1_sb[:, 0, :],
                             start=True, stop=False)
            nc.tensor.matmul(ups[:], lhsT=zt[:, 1, :], rhs=fc1_sb[:, 1, :],
                             start=False, stop=False)
            nc.tensor.matmul(ups[:], lhsT=attr_sb[:], rhs=fc1_sb[:, 2, :],
                             start=False, stop=True)
            r = sbw.tile([G, H], F32, tag="r")
            nc.scalar.activation(out=r[:], in_=ups[:], func=AF.Relu)
            wps = ptail.tile([G, H], F32, tag="tail")
            nc.tensor.matmul(wps[:], lhsT=ones_sb[:], rhs=fcw2_sb[:],
                             start=True, stop=True)
            rr = sbw.tile([G, H], F32, tag="rr")
            nc.vector.tensor_mul(out=rr[:], in0=r[:], in1=wps[:])
            o = small.tile([G, 1], F32, tag="o")
            nc.vector.tensor_reduce(out=o[:], in_=rr[:],
                                    axis=mybir.AxisListType.X, op=OP.add)
            nc.vector.tensor_scalar_add(out=o[:], in0=o[:],
                                        scalar1=fcb2_sb[:])
            nc.sync.dma_start(out=out_ap[:], in_=o[:])

    nc.compile()
    return nc


def make_in_maps(cfg, inputs, per_core, msg_bf16=False):
    """Build the per-core input maps from full inputs + preprocessed arrays."""
    H, G, A = cfg.H, cfg.G, cfg.A
    f = lambda a: np.ascontiguousarray(np.asarray(a, np.float32))
    W1, b1 = f(inputs["W1"]), f(inputs["b1"])
    W2, b2 = f(inputs["W2"]), f(inputs["b2"])
    W3, b3 = f(inputs["W3"]), f(inputs["b3"])
    g1, be1 = f(inputs["g1"]), f(inputs["be1"])
    g2, be2 = f(inputs["g2"]), f(inputs["be2"])
    g3, be3 = f(inputs["g3"]), f(inputs["be3"])
    fcW1, fcb1 = f(inputs["fcW1"]), f(inputs["fcb1"])
    fcW2, fcb2 = f(inputs["fcW2"]), f(inputs["fcb2"])
    graph_attr = f(inputs["graph_attr"]).reshape(-1, A)
    batch = np.asarray(inputs["batch"], np.int64)

    bgb = np.zeros((P, 9, H), np.float32)
    for i, v in enumerate([b1, g1, be1, b2, g2, be2, b3, g3, be3]):
        bgb[:, i, :] = v[None, :]
    fc1aug = np.zeros((3 * P, H), np.float32)
    fc1aug[:H, :] = fcW1[:H, :]
    fc1aug[2 * P:2 * P + A, :] = fcW1[H:H + A, :]
    fc1aug[2 * P + A, :] = fcb1
    attraug = np.zeros((P, G), np.float32)
    attraug[:A, :] = graph_attr.T
    attraug[A, :] = 1.0
    cnt = np.bincount(batch, minlength=G).astype(np.float32)
    invc = (1.0 / np.maximum(cnt, 1.0)).reshape(G, 1).astype(np.float32)
    fcw2row = fcW2[:, 0].reshape(1, H).copy()
    fcb2col = np.full((G, 1), fcb2[0], np.float32)
    iota_in = np.tile(np.arange(P, dtype=np.float32), (P, 1))
    if msg_bf16:
        import ml_dtypes
        iota_in = iota_in.astype(ml_dtypes.bfloat16)
    ident_in = np.eye(P, dtype=np.float32)

    shared = dict(bgb=bgb, iota_in=iota_in, ident_in=ident_in, W1=W1, W2=W2,
                  W3=W3, fc1aug=fc1aug, attraug=attraug, invc=invc,
                  fcw2row=fcw2row, fcb2col=fcb2col)
    in_maps = []
    for c in range(cfg.M):
        m = dict(shared)
        m.update(per_core[c])
        if msg_bf16:
            import ml_dtypes
            m["dslot"] = m["dslot"].astype(ml_dtypes.bfloat16)
        in_maps.append(m)
    return in_maps


_CACHE = {}


def _get_program(cfg, meta, gamma_trivial, beta_trivial, msg_bf16=False):
    key = (tuple(meta["J"].reshape(-1).tolist()), gamma_trivial, beta_trivial,
           msg_bf16)
    if key not in _CACHE:
        _CACHE[key] = build_program(cfg, meta, gamma_trivial, beta_trivial,
                                    msg_bf16)
    return _CACHE[key]


def run(cfg, inputs, nc=None, return_nc=False, msg_bf16=False):
    per_core, meta = preprocess(cfg, inputs["x"], inputs["edge_index"],
                                inputs["batch"])
    gamma_trivial = all(np.allclose(np.asarray(inputs[k]), 1.0)
                        for k in ("g1", "g2", "g3"))
    beta_trivial = all(np.allclose(np.asarray(inputs[k]), 0.0)
                       for k in ("be1", "be2", "be3"))
    if nc is None:
        nc = _get_program(cfg, meta, gamma_trivial, beta_trivial, msg_bf16)
    in_maps = make_in_maps(cfg, inputs, per_core, msg_bf16)
    res = None
    for attempt in range(3):
        try:
            res = run_bass_kernel_spmd(nc, in_maps, list(range(cfg.M)))
            break
        except Exception:
            # a previously crashed run can leave the device unrecoverable
            # for exactly one execution; retry clears it
            if attempt == 2:
                raise
    out = res.results[0]["out"].astype(np.float32)
    if return_nc:
        return out, nc
    return out


def kernel(**inputs) -> np.ndarray:
    return run(CFG, inputs)


# revision 17
# speedup vs baseline: 190.6823x; 1.0866x over previous
"""GCN model (3x GCNConv + LayerNorm + ReLU, mean-pool, 2-layer MLP head)
as a Bass SPMD kernel on 8 Trainium2 NeuronCores.

Sharding: nodes (and their incident edges, keyed by dst) are partitioned into
8 contiguous blocks. Each core computes y = dinv * (x @ W) for its block,
an AllGather replicates y, then each core aggregates messages for its dst
block with dma_gather + selection-matrix matmuls, applies LayerNorm + ReLU,
and finally segment-mean-pool partial sums are AllReduced before a tiny MLP.
"""

import math

import numpy as np

import concourse.bass as bass
import concourse.bacc as bacc
import concourse.tile as tile
import concourse.mybir as mybir
from concourse.bass_utils import run_bass_kernel_spmd
from concourse.library_config import mlp as mlp_lib

F32 = mybir.dt.float32
I16 = mybir.dt.int16
AF = mybir.ActivationFunctionType
OP = mybir.AluOpType

P = 128


class GCNConfig:
    def __init__(self, N=50000, E=800000, F_IN=128, H=256, G=64, A=8, OUT=1,
                 M=8):
        assert F_IN % P == 0 and H % P == 0
        self.N, self.E, self.F_IN, self.H, self.G, self.A, self.OUT, self.M = \
            N, E, F_IN, H, G, A, OUT, M
        self.NL = N // M                      # nodes per core (unpadded)
        assert self.NL * M == N
        self.T = (self.NL + P - 1) // P       # dst tiles per core
        self.NP = self.T * P                  # padded nodes per core
        self.NT = self.NP * M                 # padded total rows in y_full
        assert self.NT % 2 == 0
        self.HALF = self.NT // 2              # half-table rows (int16 range)
        assert self.HALF < 32768
        assert self.NP * (M // 2) == self.HALF  # halves align to core blocks
        self.KH = H // P                      # k-tiles per matmul (and F_IN)
        self.KF = F_IN // P


CFG = GCNConfig()


def _wrap_idx16(vals):
    """[n] int16 -> [128, n//16] in the 16-partition-wrapped, 8x-replicated
    layout dma_gather expects (element i at [i % 16, i // 16])."""
    n = vals.shape[0]
    assert n % 16 == 0
    arr = vals.reshape(n // 16, 16).T.astype(np.int16)   # [16, n/16]
    return np.tile(arr, (8, 1))                          # [128, n/16]


def preprocess(cfg, x, edge_index, batch):
    """Host-side sharding: per-core gather indices, dst slots, norm factors,
    pooling matrices. Returns (per_core dict list, meta dict)."""
    N, E, M, NL, NP, T, HALF = (cfg.N, cfg.E, cfg.M, cfg.NL, cfg.NP, cfg.T,
                                cfg.HALF)
    src = np.asarray(edge_index[0], dtype=np.int64)
    dst = np.asarray(edge_index[1], dtype=np.int64)
    batch = np.asarray(batch, dtype=np.int64)

    deg = np.bincount(dst, minlength=N).astype(np.float64) + 1.0
    dinv = (1.0 / np.sqrt(deg)).astype(np.float32)

    # self-loops as ordinary edges
    loop = np.arange(N, dtype=np.int64)
    src2 = np.concatenate([src, loop])
    dst2 = np.concatenate([dst, loop])

    core = dst2 // NL
    dloc = dst2 % NL
    tile_id = dloc // P
    dslot_v = (dloc % P).astype(np.float32)
    srcp = (src2 // NL) * NP + (src2 % NL)          # padded global row
    half = (srcp >= HALF).astype(np.int64)
    idx16v = (srcp - HALF * half).astype(np.int64)

    # sort key: (core, tile, half) -> contiguous blocks
    key = ((core * T + tile_id) * 2 + half)
    order = np.argsort(key, kind="stable")
    key_s = key[order]
    idx16_s = idx16v[order]
    dslot_s = dslot_v[order]

    nkeys = M * T * 2
    counts = np.bincount(key_s, minlength=nkeys).reshape(M, T, 2)
    starts = np.zeros(nkeys + 1, dtype=np.int64)
    np.cumsum(counts.reshape(-1), out=starts[1:])

    # capacities shared across cores (program is SPMD)
    J = np.ceil(counts.max(axis=0) / P).astype(np.int64)     # [T, 2]
    JMAX = int(J.sum(axis=1).max())

    # pack per-core arrays
    W16 = int(J.sum() * 8)        # idx16 free-dim (int16 cols)
    WD = int(J.sum())             # dslot free-dim (f32 cols, J per block)
    per_core = []
    for c in range(M):
        idx16 = np.zeros((P, W16), np.int16)
        dslot = np.full((P, WD), 300.0, np.float32)
        o16 = 0
        od = 0
        for t in range(T):
            for h in range(2):
                Jth = int(J[t, h])
                if Jth == 0:
                    continue
                Cap = Jth * P
                k = (c * T + t) * 2 + h
                s, e = starts[k], starts[k + 1]
                n = e - s
                vi = np.zeros(Cap, np.int64)
                vd = np.full(Cap, 300.0, np.float32)
                vi[:n] = idx16_s[s:e]
                vd[:n] = dslot_s[s:e]
                idx16[:, o16:o16 + Jth * 8] = _wrap_idx16(vi)
                dslot[:, od:od + Jth] = vd.reshape(Jth, P).T
                o16 += Jth * 8
                od += Jth
        # dinv (padded, pad rows -> 0 so pad y rows are exactly zero)
        dpad = np.zeros(NP, np.float32)
        dpad[:NL] = dinv[c * NL:(c + 1) * NL]
        dinvT = dpad.reshape(T, P).T.copy()              # [128, T]
        # pooling one-hot [128, T, G]
        pm = np.zeros((NP, cfg.G), np.float32)
        pm[np.arange(NL), batch[c * NL:(c + 1) * NL]] = 1.0
        poolm = pm.reshape(T, P, cfg.G).transpose(1, 0, 2).copy()
        # x^T slab [F_IN, NP]
        xT = np.zeros((cfg.F_IN, NP), np.float32)
        xT[:, :NL] = np.asarray(x[c * NL:(c + 1) * NL], np.float32).T
        per_core.append(dict(idx16=idx16, dslot=dslot, dinvT=dinvT,
                             poolm=poolm.reshape(P, T * cfg.G), xT=xT))

    meta = dict(J=J, JMAX=JMAX, W16=W16, WD=WD)
    return per_core, meta


def build_program(cfg, meta, gamma_trivial, beta_trivial, msg_bf16=False):
    MD = mybir.dt.bfloat16 if msg_bf16 else F32
    N, M, T, NP, NT, HALF, H, G, A = (cfg.N, cfg.M, cfg.T, cfg.NP, cfg.NT,
                                      cfg.HALF, cfg.H, cfg.G, cfg.A)
    J = meta["J"]
    JMAX = meta["JMAX"]
    KH, KF = cfg.KH, cfg.KF

    nc = bacc.Bacc("TRN2", target_bir_lowering=False, debug=False,
                   num_devices=M, num_swdge_queues=2)

    def din(name, shape, dt=F32):
        return nc.dram_tensor(name, shape, dt, kind="ExternalInput").ap()

    xT_ap = din("xT", [cfg.F_IN, NP])
    idx16_ap = din("idx16", [P, meta["W16"]], I16)
    dslot_ap = din("dslot", [P, meta["WD"]], MD)
    dinvT_ap = din("dinvT", [P, T])
    poolm_ap = din("poolm", [P, T * G])
    bgb_ap = din("bgb", [P, 9, H])
    iota_ap = din("iota_in", [P, JMAX * P], MD)
    ident_ap = din("ident_in", [P, P])
    W1_ap = din("W1", [cfg.F_IN, H])
    W2_ap = din("W2", [H, H])
    W3_ap = din("W3", [H, H])
    fc1_ap = din("fc1aug", [3 * P, H])
    attr_ap = din("attraug", [P, G])
    invc_ap = din("invc", [G, 1])
    fcw2_ap = din("fcw2row", [1, H])
    fcb2_ap = din("fcb2col", [G, 1])
    out_ap = nc.dram_tensor("out", [G, cfg.OUT], F32,
                            kind="ExternalOutput").ap()

    y_cc = nc.dram_tensor("y_cc", [NP, H], MD)
    y_full = nc.dram_tensor("y_full", [NT, H], MD, addr_space="Shared")
    pool_in = nc.dram_tensor("pool_in", [G, H], F32)
    pool_out = nc.dram_tensor("pool_out", [G, H], F32, addr_space="Shared")

    rg = [list(range(M))]

    with tile.TileContext(nc) as tc:
        with tc.tile_pool(name="const", bufs=1) as cst, \
             tc.tile_pool(name="sbw", bufs=3) as sbw, \
             tc.tile_pool(name="msgp", bufs=2) as msgp, \
             tc.tile_pool(name="sp", bufs=2) as sp, \
             tc.tile_pool(name="small", bufs=8) as small, \
             tc.tile_pool(name="psp_a", bufs=2, space="PSUM") as psp_a, \
             tc.tile_pool(name="psp_y", bufs=2, space="PSUM") as psp_y, \
             tc.tile_pool(name="pst", bufs=2, space="PSUM") as pst, \
             tc.tile_pool(name="ptail", bufs=2, space="PSUM") as ptail:

            nc.gpsimd.load_library(mlp_lib)

            # ---- constants
            idx16_sb = cst.tile([P, meta["W16"]], I16)
            nc.sync.dma_start(out=idx16_sb[:], in_=idx16_ap[:])
            dslot_sb = cst.tile([P, meta["WD"]], MD)
            nc.sync.dma_start(out=dslot_sb[:], in_=dslot_ap[:])
            dinv_sb = cst.tile([P, T], F32)
            nc.sync.dma_start(out=dinv_sb[:], in_=dinvT_ap[:])
            poolm_sb = cst.tile([P, T, G], F32)
            nc.sync.dma_start(out=poolm_sb[:],
                              in_=poolm_ap[:].rearrange("p (t g) -> p t g", g=G))
            bgb_sb = cst.tile([P, 9, H], F32)
            nc.sync.dma_start(out=bgb_sb[:], in_=bgb_ap[:])
            iota_sb = cst.tile([P, JMAX * P], MD)
            nc.sync.dma_start(out=iota_sb[:], in_=iota_ap[:])
            ident_sb = cst.tile([P, P], F32)
            nc.sync.dma_start(out=ident_sb[:], in_=ident_ap[:])
            W1_sb = cst.tile([cfg.F_IN, H], F32)
            nc.sync.dma_start(out=W1_sb[:], in_=W1_ap[:])
            W2_sb = cst.tile([P, KH, H], F32)
            nc.sync.dma_start(out=W2_sb[:],
                              in_=W2_ap[:].rearrange("(k p) h -> p k h", p=P))
            W3_sb = cst.tile([P, KH, H], F32)
            nc.sync.dma_start(out=W3_sb[:],
                              in_=W3_ap[:].rearrange("(k p) h -> p k h", p=P))
            fc1_sb = cst.tile([P, 3, H], F32)
            nc.sync.dma_start(out=fc1_sb[:],
                              in_=fc1_ap[:].rearrange("(k p) h -> p k h", p=P))
            attr_sb = cst.tile([P, G], F32)
            nc.sync.dma_start(out=attr_sb[:], in_=attr_ap[:])
            invc_sb = cst.tile([G, 1], F32)
            nc.sync.dma_start(out=invc_sb[:], in_=invc_ap[:])
            fcw2_sb = cst.tile([1, H], F32)
            nc.sync.dma_start(out=fcw2_sb[:], in_=fcw2_ap[:])
            fcb2_sb = cst.tile([G, 1], F32)
            nc.sync.dma_start(out=fcb2_sb[:], in_=fcb2_ap[:])

            eps_sb = cst.tile([P, 1], F32)
            nc.vector.memset(eps_sb[:], 1e-5)
            ones_sb = cst.tile([1, G], F32)
            nc.vector.memset(ones_sb[:], 1.0)
            pool_acc = cst.tile([G, H], F32)
            nc.vector.memset(pool_acc[:], 0.0)
            hT_sb = cst.tile([P, T * KH, P], F32)

            # gpsimd registers for num_idxs (reuse per distinct value)
            regs = {}
            for v in sorted({int(J[t, h]) * P for t in range(T)
                             for h in range(2) if J[t, h] > 0}):
                regs[v] = nc.gpsimd.to_reg(v)

            # block offsets into idx16/dslot slabs
            o16 = np.zeros((T, 2), np.int64)
            od = np.zeros((T, 2), np.int64)
            acc16 = 0
            accd = 0
            for t in range(T):
                for h in range(2):
                    o16[t, h] = acc16
                    od[t, h] = accd
                    acc16 += int(J[t, h]) * 8
                    accd += int(J[t, h])

            Wsb = [W1_sb, W2_sb, W3_sb]

            def emit_phase_a(L, t):
                # y_L tile t = dinv * (h_{L-1} @ W_L); layer 0 reads xT input
                psy = psp_y.tile([P, H], F32, tag="psy")
                if L == 0:
                    xt = sbw.tile([P, P], F32, tag="xt")
                    nc.sync.dma_start(out=xt[:],
                                      in_=xT_ap[:, t * P:(t + 1) * P])
                    nc.tensor.matmul(psy[:], lhsT=xt[:], rhs=W1_sb[:],
                                     start=True, stop=True)
                else:
                    for kk in range(KH):
                        nc.tensor.matmul(
                            psy[:], lhsT=hT_sb[:, t * KH + kk, :],
                            rhs=Wsb[L][:, kk, :],
                            start=(kk == 0), stop=(kk == KH - 1))
                ysb = sbw.tile([P, H], MD, tag="y")
                nc.scalar.mul(out=ysb[:], in_=psy[:],
                              mul=dinv_sb[:, t:t + 1])
                nc.sync.dma_start(out=y_cc[t * P:(t + 1) * P, :], in_=ysb[:])

            def emit_allgather():
                nc.gpsimd.collective_compute(
                    "AllGather", OP.bypass, replica_groups=rg,
                    ins=[y_cc[:]], outs=[y_full[:]])

            for t in range(T):
                emit_phase_a(0, t)
            emit_allgather()

            for L in range(3):
                # ---------- aggregation per dst tile (next layer's phase A
                # interleaved per tile so the next AllGather starts as soon
                # as the last tile's aggregation lands) ----------
                for t in range(T):
                    Jl, Jh = int(J[t, 0]), int(J[t, 1])
                    Jt = Jl + Jh
                    msg = msgp.tile([P, JMAX, H], MD, tag="msg")
                    if Jl > 0:
                        nc.gpsimd.dma_gather(
                            out_ap=msg[:, :Jl, :], in_ap=y_full[:HALF, :],
                            idxs_ap=idx16_sb[:, int(o16[t, 0]):int(o16[t, 0]) + Jl * 8],
                            num_idxs=Jl * P, num_idxs_reg=regs[Jl * P],
                            elem_size=H, single_packet=False,
                            queue_num=t % 2)
                    if Jh > 0:
                        nc.gpsimd.dma_gather(
                            out_ap=msg[:, Jl:Jt, :], in_ap=y_full[HALF:, :],
                            idxs_ap=idx16_sb[:, int(o16[t, 1]):int(o16[t, 1]) + Jh * 8],
                            num_idxs=Jh * P, num_idxs_reg=regs[Jh * P],
                            elem_size=H, single_packet=False,
                            queue_num=t % 2)
                    ps = psp_a.tile([P, H], F32, tag="agg")
                    # one wide selection-matrix build for all chunks of this
                    # tile (lo+hi dslot blocks are adjacent in the slab)
                    a = int(od[t, 0])
                    S_all = sp.tile([P, JMAX, P], MD, tag="S")
                    dsl = dslot_sb[:, a:a + Jt]
                    dsl_b = bass.AP(tensor=dsl.tensor, offset=dsl.offset,
                                    ap=[dsl.ap[0], dsl.ap[1], [0, P]])
                    nc.vector.tensor_tensor(
                        out=S_all[:, :Jt, :],
                        in0=iota_sb[:, :Jt * P].rearrange(
                            "p (j d) -> p j d", d=P),
                        in1=dsl_b,
                        op=OP.is_equal)
                    for j in range(Jt):
                        nc.tensor.matmul(ps[:], lhsT=S_all[:, j, :],
                                         rhs=msg[:, j, :],
                                         start=(j == 0), stop=(j == Jt - 1))

                    # ---------- evict + bias + LN + relu ----------
                    tt = sbw.tile([P, H], F32, tag="tt")
                    nc.vector.tensor_scalar_mul(out=tt[:], in0=ps[:],
                                                scalar1=dinv_sb[:, t:t + 1])
                    nc.vector.tensor_add(out=tt[:], in0=tt[:],
                                         in1=bgb_sb[:, 3 * L + 0, :])
                    stats = small.tile([P, 6], F32, tag="stats")
                    nc.vector.bn_stats(out=stats[:], in_=tt[:])
                    mv = small.tile([P, 2], F32, tag="mv")
                    nc.vector.bn_aggr(out=mv[:], in_=stats[:])
                    rstd = small.tile([P, 1], F32, tag="rstd")
                    nc.scalar.activation(out=rstd[:], in_=mv[:, 1:2],
                                         func=AF.Sqrt, bias=eps_sb[:],
                                         scale=1.0)
                    nc.vector.reciprocal(out=rstd[:], in_=rstd[:])
                    nc.vector.tensor_scalar(
                        out=tt[:], in0=tt[:], scalar1=mv[:, 0:1],
                        scalar2=rstd[:], op0=OP.subtract, op1=OP.mult)
                    if not gamma_trivial:
                        nc.vector.tensor_mul(out=tt[:], in0=tt[:],
                                             in1=bgb_sb[:, 3 * L + 1, :])
                    if not beta_trivial:
                        nc.vector.tensor_add(out=tt[:], in0=tt[:],
                                             in1=bgb_sb[:, 3 * L + 2, :])
                    h_t = sbw.tile([P, H], F32, tag="h")
                    nc.scalar.activation(out=h_t[:], in_=tt[:], func=AF.Relu)

                    if L < 2:
                        for kk in range(KH):
                            pt = pst.tile([P, P], F32, tag="pt")
                            nc.tensor.transpose(
                                out=pt[:], in_=h_t[:, kk * P:(kk + 1) * P],
                                identity=ident_sb[:])
                            nc.vector.tensor_copy(
                                out=hT_sb[:, t * KH + kk, :], in_=pt[:])
                        emit_phase_a(L + 1, t)
                    else:
                        pp = ptail.tile([G, H], F32, tag="tail")
                        nc.tensor.matmul(pp[:], lhsT=poolm_sb[:, t, :],
                                         rhs=h_t[:], start=True, stop=True)
                        nc.vector.tensor_add(out=pool_acc[:], in0=pool_acc[:],
                                             in1=pp[:])
                if L < 2:
                    emit_allgather()

            # ---------- pooled mean + MLP head ----------
            nc.sync.dma_start(out=pool_in[:], in_=pool_acc[:])
            nc.gpsimd.collective_compute(
                "AllReduce", OP.add, replica_groups=rg,
                ins=[pool_in[:]], outs=[pool_out[:]])
            pooled = sbw.tile([G, H], F32, tag="pooled")
            nc.sync.dma_start(out=pooled[:], in_=pool_out[:])
            nc.vector.tensor_scalar_mul(out=pooled[:], in0=pooled[:],
                                        scalar1=invc_sb[:])
            zt = sbw.tile([P, KH, G], F32, tag="zt")
            for kk in range(KH):
                pz = ptail.tile([P, G], F32, tag="tail")
                nc.tensor.transpose(out=pz[:], in_=pooled[:, kk * P:(kk + 1) * P],
                                    identity=ident_sb[:G, :G])
                nc.vector.tensor_copy(out=zt[:, kk, :], in_=pz[:])
            ups = ptail.tile([G, H], F32, tag="tail")
            nc.tensor.matmul(ups[:], lhsT=zt[:, 0, :], rhs=fc1_sb[:, 0, :],
                             start=True, stop=False)
            nc.tensor.matmul(ups[:], lhsT=zt[:, 1, :], rhs=fc1_sb[:, 1, :],
                             start=False, stop=False)
            nc.tensor.matmul(ups[:], lhsT=attr_sb[:], rhs=fc1_sb[:, 2, :],
                             start=False, stop=True)
            r = sbw.tile([G, H], F32, tag="r")
            nc.scalar.activation(out=r[:], in_=ups[:], func=AF.Relu)
            wps = ptail.tile([G, H], F32, tag="tail")
            nc.tensor.matmul(wps[:], lhsT=ones_sb[:], rhs=fcw2_sb[:],
                             start=True, stop=True)
            rr = sbw.tile([G, H], F32, tag="rr")
            nc.vector.tensor_mul(out=rr[:], in0=r[:], in1=wps[:])
            o = small.tile([G, 1], F32, tag="o")
            nc.vector.tensor_reduce(out=o[:], in_=rr[:],
                                    axis=mybir.AxisListType.X, op=OP.add)
            nc.vector.tensor_scalar_add(out=o[:], in0=o[:],
                                        scalar1=fcb2_sb[:])
            nc.sync.dma_start(out=out_ap[:], in_=o[:])

    nc.compile()
    return nc


def make_in_maps(cfg, inputs, per_core, msg_bf16=False, jmax=1, meta=None):
    """Build the per-core input maps from full inputs + preprocessed arrays."""
    H, G, A = cfg.H, cfg.G, cfg.A
    f = lambda a: np.ascontiguousarray(np.asarray(a, np.float32))
    W1, b1 = f(inputs["W1"]), f(inputs["b1"])
    W2, b2 = f(inputs["W2"]), f(inputs["b2"])
    W3, b3 = f(inputs["W3"]), f(inputs["b3"])
    g1, be1 = f(inputs["g1"]), f(inputs["be1"])
    g2, be2 = f(inputs["g2"]), f(inputs["be2"])
    g3, be3 = f(inputs["g3"]), f(inputs["be3"])
    fcW1, fcb1 = f(inputs["fcW1"]), f(inputs["fcb1"])
    fcW2, fcb2 = f(inputs["fcW2"]), f(inputs["fcb2"])
    graph_attr = f(inputs["graph_attr"]).reshape(-1, A)
    batch = np.asarray(inputs["batch"], np.int64)

    bgb = np.zeros((P, 9, H), np.float32)
    for i, v in enumerate([b1, g1, be1, b2, g2, be2, b3, g3, be3]):
        bgb[:, i, :] = v[None, :]
    fc1aug = np.zeros((3 * P, H), np.float32)
    fc1aug[:H, :] = fcW1[:H, :]
    fc1aug[2 * P:2 * P + A, :] = fcW1[H:H + A, :]
    fc1aug[2 * P + A, :] = fcb1
    attraug = np.zeros((P, G), np.float32)
    attraug[:A, :] = graph_attr.T
    attraug[A, :] = 1.0
    cnt = np.bincount(batch, minlength=G).astype(np.float32)
    invc = (1.0 / np.maximum(cnt, 1.0)).reshape(G, 1).astype(np.float32)
    fcw2row = fcW2[:, 0].reshape(1, H).copy()
    fcb2col = np.full((G, 1), fcb2[0], np.float32)
    iota_in = np.tile(np.arange(P, dtype=np.float32), (P, jmax))
    if msg_bf16:
        import ml_dtypes
        iota_in = iota_in.astype(ml_dtypes.bfloat16)
    ident_in = np.eye(P, dtype=np.float32)

    shared = dict(bgb=bgb, iota_in=iota_in, ident_in=ident_in, W1=W1, W2=W2,
                  W3=W3, fc1aug=fc1aug, attraug=attraug, invc=invc,
                  fcw2row=fcw2row, fcb2col=fcb2col)
    in_maps = []
    for c in range(cfg.M):
        m = dict(shared)
        m.update(per_core[c])
        if msg_bf16:
            import ml_dtypes
            m["dslot"] = m["dslot"].astype(ml_dtypes.bfloat16)
        in_maps.append(m)
    return in_maps


_CACHE = {}


def _get_program(cfg, meta, gamma_trivial, beta_trivial, msg_bf16=False):
    key = (tuple(meta["J"].reshape(-1).tolist()), gamma_trivial, beta_trivial,
           msg_bf16)
    if key not in _CACHE:
        _CACHE[key] = build_program(cfg, meta, gamma_trivial, beta_trivial,
                                    msg_bf16)
    return _CACHE[key]


def run(cfg, inputs, nc=None, return_nc=False, msg_bf16=False):
    per_core, meta = preprocess(cfg, inputs["x"], inputs["edge_index"],
                                inputs["batch"])
    gamma_trivial = all(np.allclose(np.asarray(inputs[k]), 1.0)
                        for k in ("g1", "g2", "g3"))
    beta_trivial = all(np.allclose(np.asarray(inputs[k]), 0.0)
                       for k in ("be1", "be2", "be3"))
    if nc is None:
        nc = _get_program(cfg, meta, gamma_trivial, beta_trivial, msg_bf16)
    in_maps = make_in_maps(cfg, inputs, per_core, msg_bf16, meta["JMAX"])
    res = None
    for attempt in range(3):
        try:
            res = run_bass_kernel_spmd(nc, in_maps, list(range(cfg.M)))
            break
        except Exception:
            # a previously crashed run can leave the device unrecoverable
            # for exactly one execution; retry clears it
            if attempt == 2:
                raise
    out = res.results[0]["out"].astype(np.float32)
    if return_nc:
        return out, nc
    return out


def kernel(**inputs) -> np.ndarray:
    return run(CFG, inputs)
